# revision 20
# baseline (speedup 1.0000x reference)
"""Luong attention kernel for Trainium2 (Bass/Tile), batch-parallel over 8 NeuronCores.

Problem (per full input):
    enc_mask [64, 2048] bool, enc_out [64, 2048, 1024] f32, dec_hid [64, 1024] f32
    sims    = einsum('bsd,bd->bs', enc_out, dec_hid); masked -> -inf
    attn    = softmax(sims, axis=1)
    context = einsum('bs,bsd->bd', attn, enc_out)

Strategy: pure data parallelism -- batch dim 64 split 8 ways (8 examples per
core).  Per core, enc_out (64 MB) is streamed through SBUF exactly once on a
dedicated DMA queue (SP/HWDGE) that carries nothing else, so the stream never
stalls behind dependent transfers.  Per 128-row chunk of s:

  * einsum1 (contract d): ONE fused DVE tensor_tensor_reduce per chunk --
    prod_bf (bf16 product, einsum2's moving operand) and the masked score sum
    (mask folded in as the reduction's initial value) come out of a single
    pass.  No second bulk pass on any engine.
  * softmax: DVE free-dim max; GPSIMD partition all-reduce + negate; ScalarE
    Exp with bf16 main output (einsum2's stationary operand) and fused sum;
    GPSIMD all-reduce.
  * einsum2 (contract s): TensorE matmuls accumulated in PSUM; epilogue
    (1/L scale on ScalarE + store on the ACT DMA queue) is deferred one
    example so no engine ever waits on TensorE.

The device computes sum_s w*(enc*dec) = dec .* context; a host-side divide
undoes the dec factor.  s is laid out as s = p*CH + c (p = SBUF partition,
c = chunk) so every DMA is fully contiguous per partition.
"""

from contextlib import ExitStack

import numpy as np

import concourse.bacc as bacc
import concourse.bass as bass
import concourse.tile as tile
from concourse import bass_isa, library_config, mybir
from concourse.bass_utils import run_bass_kernel_spmd

B, S, D = 64, 2048, 1024
N_CORES = 8
BPC = B // N_CORES  # examples per core
P = 128  # SBUF partitions

NEG_BIG = -1.0e30


def build_kernel_body(ctx: ExitStack, tc: "tile.TileContext", enc, msk, dec, out,
                      bpc: int, s: int, d: int, dq: int = 2):
    nc = tc.nc
    ch = s // P                     # chunks of 128 s-values per example
    n_dma = ch // dq                # enc DMAs per example
    d_segs = [(h, min(512, d - h)) for h in range(0, d, 512)]

    encp = ctx.enter_context(tc.tile_pool(name="encp", bufs=8))
    prodp = ctx.enter_context(tc.tile_pool(name="prodp", bufs=2))
    decp = ctx.enter_context(tc.tile_pool(name="decp", bufs=3))
    smallp = ctx.enter_context(tc.tile_pool(name="smallp", bufs=2))
    outp = ctx.enter_context(tc.tile_pool(name="outp", bufs=2))
    psum_b = ctx.enter_context(tc.tile_pool(name="psum_b", bufs=2, space="PSUM"))
    psum_c = ctx.enter_context(tc.tile_pool(name="psum_c", bufs=2, space="PSUM"))

    nc.gpsimd.load_library(library_config.attnmlp)

    ones = smallp.tile([1, P], mybir.dt.float32, bufs=1)
    nc.vector.memset(ones, 1.0)

    # ---- masks for all examples: [128, bpc, ch] {0,1} -> {0, -1e30}
    # Preamble DMAs go on the SP queue BEFORE the enc stream: small qAct
    # transfers starve behind the deep qSP backlog of 2 MiB enc DMAs.
    mask_all = smallp.tile([P, bpc, ch], mybir.dt.uint8, tag="mask_all", bufs=1)
    nc.sync.dma_start(out=mask_all, in_=msk.rearrange("b (p c) -> p b c", p=P))
    maskneg_all = smallp.tile([P, bpc, ch], mybir.dt.float32, tag="maskneg_all",
                              bufs=1)
    nc.vector.tensor_scalar_mul(maskneg_all, mask_all, NEG_BIG)

    def load_dec(b, q=None):
        """dec row b broadcast to all 128 partitions (ones-matmul on TensorE,
        PSUM drained to SBUF on the otherwise-idle ScalarE)."""
        dec_row = decp.tile([1, d], mybir.dt.float32, tag="dec_row")
        (q or nc.scalar).dma_start(out=dec_row, in_=dec[b : b + 1, :])
        dec_ps = psum_b.tile([P, d], mybir.dt.float32, tag="dec_ps")
        for h0, hw in d_segs:
            nc.tensor.matmul(dec_ps[:, h0 : h0 + hw], lhsT=ones,
                             rhs=dec_row[:, h0 : h0 + hw], start=True, stop=True)
        dec_b = decp.tile([P, d], mybir.dt.float32, tag="dec_b")
        nc.scalar.activation(dec_b, dec_ps, mybir.ActivationFunctionType.Copy)
        return dec_b

    dec_tiles = {0: load_dec(0, q=nc.sync)}
    if bpc > 1:
        dec_tiles[1] = load_dec(1, q=nc.sync)

    # deferred per-example tail work, flushed at fixed points inside the NEXT
    # example's chunk loop so no engine's in-order stream ever blocks
    pending = {}

    def flush_einsum2():
        ctxps = psum_c.tile([1, d], mybir.dt.float32, tag="ctxps")
        for c in range(ch):
            for h0, hw in d_segs:
                nc.tensor.matmul(
                    ctxps[:, h0 : h0 + hw],
                    lhsT=pending["expw_bf"][:, c : c + 1],
                    rhs=pending["prod_bf"][:, c, h0 : h0 + hw],
                    start=(c == 0),
                    stop=(c == ch - 1),
                )
        pending["ctxps"] = ctxps

    def flush_recip():
        invl = smallp.tile([P, 1], mybir.dt.float32, tag="invl")
        nc.vector.reciprocal(invl, pending["lsum"])
        pending["invl"] = invl

    def flush_epilogue():
        b_ = pending["b"]
        ctx_sb = outp.tile([1, d], mybir.dt.float32, tag="ctx_sb")
        nc.scalar.activation(ctx_sb, pending["ctxps"],
                             mybir.ActivationFunctionType.Copy,
                             scale=pending["invl"][0:1, :])
        nc.scalar.dma_start(out=out[b_ : b_ + 1, :], in_=ctx_sb)
        pending.clear()

    def softmax_span(sims_raw, b, c0, nch, tag):
        """mask + max + all-reduce + negate + exp for chunks [c0, c0+nch)."""
        simsh = smallp.tile([P, nch], mybir.dt.float32, tag=f"sims{tag}")
        nc.vector.tensor_add(simsh, sims_raw[:, c0 : c0 + nch],
                             maskneg_all[:, b, c0 : c0 + nch])
        maxcol = smallp.tile([P, 1], mybir.dt.float32, tag=f"maxcol{tag}")
        nc.vector.reduce_max(maxcol, simsh, axis=mybir.AxisListType.X)
        maxall = smallp.tile([P, 1], mybir.dt.float32, tag=f"maxall{tag}")
        nc.gpsimd.partition_all_reduce(maxall, maxcol, channels=P,
                                       reduce_op=bass_isa.ReduceOp.max)
        negmax = smallp.tile([P, 1], mybir.dt.float32, tag=f"negmax{tag}")
        nc.scalar.activation(negmax, maxall, mybir.ActivationFunctionType.Copy,
                             scale=-1.0)
        expw = smallp.tile([P, nch], mybir.dt.bfloat16, tag=f"expw{tag}")
        expsum = smallp.tile([P, 1], mybir.dt.float32, tag=f"expsum{tag}")
        nc.scalar.activation(expw, simsh, mybir.ActivationFunctionType.Exp,
                             bias=negmax, scale=1.0, accum_out=expsum)
        lsum = smallp.tile([P, 1], mybir.dt.float32, tag=f"lsum{tag}")
        nc.gpsimd.partition_all_reduce(lsum, expsum, channels=P,
                                       reduce_op=bass_isa.ReduceOp.add)
        return maxall, expw, lsum

    def einsum2_span(prod_bf, expw, c0, nch):
        ctxps = psum_c.tile([1, d], mybir.dt.float32, tag="ctxps")
        for i in range(nch):
            for h0, hw in d_segs:
                nc.tensor.matmul(
                    ctxps[:, h0 : h0 + hw],
                    lhsT=expw[:, i : i + 1],
                    rhs=prod_bf[:, c0 + i, h0 : h0 + hw],
                    start=(i == 0),
                    stop=(i == nch - 1),
                )
        return ctxps

    hh = ch // 2
    half_q = hh // dq

    for b in range(bpc):
        last = b == bpc - 1
        enc3 = enc[b].rearrange("(p c) d -> p c d", p=P)
        dec_b = dec_tiles.pop(b)
        prod_bf = prodp.tile([P, ch, d], mybir.dt.bfloat16, tag="prod_bf")
        sims_raw = smallp.tile([P, ch], mybir.dt.float32, tag="sims_raw")
        halfA = None

        for q in range(n_dma):
            if last and q == n_dma - 1 and dq > 1:
                # final group of the final example: single-chunk DMAs so the
                # DVE can start on chunk ch-2 while ch-1 is still in flight
                tiles = []
                for cc in range(dq):
                    c = q * dq + cc
                    enc_1 = encp.tile([P, 1, d], mybir.dt.float32,
                                      tag="enc_last", bufs=2)
                    nc.sync.dma_start(out=enc_1, in_=enc3[:, c : c + 1, :])
                    tiles.append(enc_1)
                for cc in range(dq):
                    c = q * dq + cc
                    nc.vector.scalar_tensor_tensor(
                        out=prod_bf[:, c, :],
                        in0=tiles[cc][:, 0, :],
                        scalar=1.0,
                        in1=dec_b,
                        op0=mybir.AluOpType.mult,
                        op1=mybir.AluOpType.mult,
                        accum_out=sims_raw[:, c : c + 1],
                    )
                continue
            enc_q = encp.tile([P, dq, d], mybir.dt.float32, tag="enc")
            nc.sync.dma_start(out=enc_q, in_=enc3[:, q * dq : (q + 1) * dq, :])
            if q == 0 and b + 2 < bpc:
                dec_tiles[b + 2] = load_dec(b + 2)
            if last and q == half_q:
                # last example: half A's softmax + context matmuls run while
                # half B is still streaming, so the end-of-kernel tail only
                # holds half B's work (and the PE clock arrives warm)
                mA, expwA, lsumA = softmax_span(sims_raw, b, 0, hh, "A")
                ctxA = einsum2_span(prod_bf, expwA, 0, hh)
                halfA = (mA, lsumA, ctxA)
            for cc in range(dq):
                c = q * dq + cc
                nc.vector.scalar_tensor_tensor(
                    out=prod_bf[:, c, :],
                    in0=enc_q[:, cc, :],
                    scalar=1.0,
                    in1=dec_b,
                    op0=mybir.AluOpType.mult,
                    op1=mybir.AluOpType.mult,
                    accum_out=sims_raw[:, c : c + 1],
                )
            if pending:
                if q >= 1 and "ctxps" not in pending:
                    flush_einsum2()
                elif q >= 3 and "invl" not in pending:
                    flush_recip()
                elif q >= 4 and "invl" in pending:
                    flush_epilogue()

        if not last:
            # ---- whole-example softmax: mask, max, exp (+bf16 weights + sum)
            maxall, expw_bf, lsum = softmax_span(sims_raw, b, 0, ch, "F")
            pending.update(b=b, prod_bf=prod_bf, expw_bf=expw_bf, lsum=lsum)

    # ---- last example's half-B tail + flash-style combine of the halves:
    # ctx = (sA*ctxA + sB*ctxB) / (sA*LA + sB*LB),  sH = exp(mH - max(mA,mB))
    b = bpc - 1
    mA, lsumA, ctxA = halfA
    mB, expwB, lsumB = softmax_span(sims_raw, b, hh, hh, "B")
    ctxB = einsum2_span(prod_bf, expwB, hh, hh)

    m = smallp.tile([P, 1], mybir.dt.float32, tag="m")
    nc.vector.tensor_max(m, mA, mB)
    negm = smallp.tile([P, 1], mybir.dt.float32, tag="negm")
    nc.scalar.activation(negm, m, mybir.ActivationFunctionType.Copy, scale=-1.0)
    sA = smallp.tile([P, 1], mybir.dt.float32, tag="sA")
    nc.scalar.activation(sA, mA, mybir.ActivationFunctionType.Exp, bias=negm)
    sB = smallp.tile([P, 1], mybir.dt.float32, tag="sB")
    nc.scalar.activation(sB, mB, mybir.ActivationFunctionType.Exp, bias=negm)
    t1 = smallp.tile([P, 1], mybir.dt.float32, tag="t1")
    nc.vector.tensor_scalar_mul(t1, lsumB, sB)
    ltot = smallp.tile([P, 1], mybir.dt.float32, tag="ltot")
    nc.vector.scalar_tensor_tensor(out=ltot, in0=lsumA, scalar=sA, in1=t1,
                                   op0=mybir.AluOpType.mult,
                                   op1=mybir.AluOpType.add)
    invt = smallp.tile([P, 1], mybir.dt.float32, tag="invt")
    nc.vector.reciprocal(invt, ltot)
    wA = smallp.tile([P, 1], mybir.dt.float32, tag="wA")
    nc.vector.tensor_scalar_mul(wA, sA, invt)
    wB = smallp.tile([P, 1], mybir.dt.float32, tag="wB")
    nc.vector.tensor_scalar_mul(wB, sB, invt)

    u = outp.tile([1, d], mybir.dt.float32, tag="u")
    nc.scalar.activation(u, ctxA, mybir.ActivationFunctionType.Copy,
                         scale=wA[0:1, :])
    ctx_sb = outp.tile([1, d], mybir.dt.float32, tag="ctx_sb_last")
    nc.vector.scalar_tensor_tensor(out=ctx_sb, in0=ctxB,
                                   scalar=wB[0:1, :], in1=u,
                                   op0=mybir.AluOpType.mult,
                                   op1=mybir.AluOpType.add)
    nc.scalar.dma_start(out=out[b : b + 1, :], in_=ctx_sb)

    # flush example bpc-2's deferred epilogue
    if pending:
        if "ctxps" not in pending:
            flush_einsum2()
        if "invl" not in pending:
            flush_recip()
        flush_epilogue()


def build_nc(bpc: int = BPC, s: int = S, d: int = D, dq: int = 2):
    nc = bacc.Bacc("TRN2", target_bir_lowering=False, debug=False)
    enc = nc.dram_tensor("enc_out", [bpc, s, d], mybir.dt.float32,
                         kind="ExternalInput").ap()
    msk = nc.dram_tensor("enc_mask", [bpc, s], mybir.dt.uint8,
                         kind="ExternalInput").ap()
    dec = nc.dram_tensor("dec_hid", [bpc, d], mybir.dt.float32,
                         kind="ExternalInput").ap()
    out = nc.dram_tensor("context", [bpc, d], mybir.dt.float32,
                         kind="ExternalOutput").ap()
    with tile.TileContext(nc) as tc, ExitStack() as ctx:
        build_kernel_body(ctx, tc, enc, msk, dec, out, bpc, s, d, dq)
    nc.compile()
    return nc


_NC_CACHE = {}


def _get_nc():
    if "nc" not in _NC_CACHE:
        _NC_CACHE["nc"] = build_nc()
    return _NC_CACHE["nc"]


def run_sharded(enc_mask, enc_out, dec_hid, trace=False, **kw):
    """Shard over batch, run on 8 cores, return (full_output, BassKernelResults)."""
    nc = _get_nc()
    enc_mask = np.ascontiguousarray(enc_mask).astype(np.uint8)
    enc_out = np.ascontiguousarray(enc_out, dtype=np.float32)
    dec_hid = np.ascontiguousarray(dec_hid, dtype=np.float32)
    in_maps = [
        {
            "enc_mask": enc_mask[i * BPC : (i + 1) * BPC],
            "enc_out": enc_out[i * BPC : (i + 1) * BPC],
            "dec_hid": dec_hid[i * BPC : (i + 1) * BPC],
        }
        for i in range(N_CORES)
    ]
    res = run_bass_kernel_spmd(nc, in_maps, core_ids=list(range(N_CORES)),
                               trace=trace, **kw)
    full = np.concatenate([r["context"] for r in res.results], axis=0)
    # The device computes sum_s w[s] * (enc[s,:]*dec) = dec .* context;
    # undo the dec factor here.
    full = full / dec_hid
    return full, res


def kernel(enc_mask, enc_out, dec_hid):
    full, _ = run_sharded(enc_mask, enc_out, dec_hid)
    return full.astype(np.float32)


# revision 21
# speedup vs baseline: 1.0920x; 1.0920x over previous
"""Luong attention kernel for Trainium2 (Bass/Tile), batch-parallel over 8 NeuronCores.

Problem (per full input):
    enc_mask [64, 2048] bool, enc_out [64, 2048, 1024] f32, dec_hid [64, 1024] f32
    sims    = einsum('bsd,bd->bs', enc_out, dec_hid); masked -> -inf
    attn    = softmax(sims, axis=1)
    context = einsum('bs,bsd->bd', attn, enc_out)

Strategy: pure data parallelism -- batch dim 64 split 8 ways (8 examples per
core).  Per core, enc_out (64 MB) is streamed through SBUF exactly once on a
dedicated DMA queue (SP/HWDGE) that carries nothing else, so the stream never
stalls behind dependent transfers.  Per 128-row chunk of s:

  * einsum1 (contract d): ONE fused DVE scalar_tensor_tensor per chunk --
    prod_bf (bf16 product, einsum2's moving operand) and the score sum
    (accum_out) come out of a single pass.  No second bulk pass on any
    engine.  (tensor_tensor_reduce would also fold the mask in, but that
    opcode wedges the HW; scalar_tensor_tensor lowers to a standard
    instruction and a tiny [128,16] mask add is done on the side.)
  * softmax: DVE free-dim max; GPSIMD partition all-reduce; ScalarE negate +
    Exp with bf16 main output (einsum2's stationary operand) and fused sum;
    GPSIMD all-reduce.
  * einsum2 (contract s): TensorE matmuls accumulated in PSUM; epilogue
    (1/L scale on ScalarE + store on the ACT DMA queue) is deferred one
    example so no engine ever waits on TensorE.
  * the LAST example is split in two halves with independent softmax stats,
    combined flash-style [ctx = (sA*ctxA + sB*ctxB)/(sA*LA + sB*LB)]: half
    A's context matmuls overlap half B's streaming, so the end-of-kernel
    tail only holds half B's work and the PE clock (HAM gate) arrives warm.

The device computes sum_s w*(enc*dec) = dec .* context; a host-side divide
undoes the dec factor.  s is laid out as s = p*CH + c (p = SBUF partition,
c = chunk) so every DMA is fully contiguous per partition.
"""

from contextlib import ExitStack

import numpy as np

import concourse.bacc as bacc
import concourse.bass as bass
import concourse.tile as tile
from concourse import bass_isa, library_config, mybir
from concourse.bass_utils import run_bass_kernel_spmd

B, S, D = 64, 2048, 1024
N_CORES = 8
BPC = B // N_CORES  # examples per core
P = 128  # SBUF partitions

NEG_BIG = -1.0e30


def build_kernel_body(ctx: ExitStack, tc: "tile.TileContext", enc, msk, dec, out,
                      bpc: int, s: int, d: int, dq: int = 2):
    nc = tc.nc
    ch = s // P                     # chunks of 128 s-values per example
    n_dma = ch // dq                # enc DMAs per example
    d_segs = [(h, min(512, d - h)) for h in range(0, d, 512)]

    encp = ctx.enter_context(tc.tile_pool(name="encp", bufs=8))
    prodp = ctx.enter_context(tc.tile_pool(name="prodp", bufs=2))
    decp = ctx.enter_context(tc.tile_pool(name="decp", bufs=3))
    smallp = ctx.enter_context(tc.tile_pool(name="smallp", bufs=2))
    outp = ctx.enter_context(tc.tile_pool(name="outp", bufs=2))
    psum_b = ctx.enter_context(tc.tile_pool(name="psum_b", bufs=2, space="PSUM"))
    psum_c = ctx.enter_context(tc.tile_pool(name="psum_c", bufs=2, space="PSUM"))

    nc.gpsimd.load_library(library_config.attnmlp)

    ones = smallp.tile([1, P], mybir.dt.float32, bufs=1)
    nc.vector.memset(ones, 1.0)

    # ---- masks for all examples: [128, bpc, ch] {0,1} -> {0, -1e30}
    # Preamble DMAs go on the SP queue BEFORE the enc stream: small qAct
    # transfers starve behind the deep qSP backlog of 2 MiB enc DMAs.
    mask_all = smallp.tile([P, bpc, ch], mybir.dt.uint8, tag="mask_all", bufs=1)
    nc.sync.dma_start(out=mask_all, in_=msk.rearrange("b (p c) -> p b c", p=P))
    maskneg_all = smallp.tile([P, bpc, ch], mybir.dt.float32, tag="maskneg_all",
                              bufs=1)
    nc.vector.tensor_scalar_mul(maskneg_all, mask_all, NEG_BIG)

    def load_dec(b, q=None):
        """dec row b broadcast to all 128 partitions (ones-matmul on TensorE,
        PSUM drained to SBUF on the otherwise-idle ScalarE)."""
        dec_row = decp.tile([1, d], mybir.dt.float32, tag="dec_row")
        (q or nc.scalar).dma_start(out=dec_row, in_=dec[b : b + 1, :])
        dec_ps = psum_b.tile([P, d], mybir.dt.float32, tag="dec_ps")
        for h0, hw in d_segs:
            nc.tensor.matmul(dec_ps[:, h0 : h0 + hw], lhsT=ones,
                             rhs=dec_row[:, h0 : h0 + hw], start=True, stop=True)
        dec_b = decp.tile([P, d], mybir.dt.float32, tag="dec_b")
        nc.scalar.activation(dec_b, dec_ps, mybir.ActivationFunctionType.Copy)
        return dec_b

    dec_tiles = {0: load_dec(0, q=nc.sync)}
    if bpc > 1:
        dec_tiles[1] = load_dec(1, q=nc.sync)

    # deferred per-example tail work, flushed at fixed points inside the NEXT
    # example's chunk loop so no engine's in-order stream ever blocks
    pending = {}

    def flush_einsum2():
        ctxps = psum_c.tile([1, d], mybir.dt.float32, tag="ctxps")
        for c in range(ch):
            for h0, hw in d_segs:
                nc.tensor.matmul(
                    ctxps[:, h0 : h0 + hw],
                    lhsT=pending["expw_bf"][:, c : c + 1],
                    rhs=pending["prod_bf"][:, c, h0 : h0 + hw],
                    start=(c == 0),
                    stop=(c == ch - 1),
                )
        pending["ctxps"] = ctxps

    def flush_recip():
        invl = smallp.tile([P, 1], mybir.dt.float32, tag="invl")
        nc.vector.reciprocal(invl, pending["lsum"])
        pending["invl"] = invl

    def flush_epilogue():
        b_ = pending["b"]
        ctx_sb = outp.tile([1, d], mybir.dt.float32, tag="ctx_sb")
        nc.scalar.activation(ctx_sb, pending["ctxps"],
                             mybir.ActivationFunctionType.Copy,
                             scale=pending["invl"][0:1, :])
        nc.scalar.dma_start(out=out[b_ : b_ + 1, :], in_=ctx_sb)
        pending.clear()

    def softmax_span(sims_raw, b, c0, nch, tag):
        """mask + max + all-reduce + negate + exp for chunks [c0, c0+nch)."""
        simsh = smallp.tile([P, nch], mybir.dt.float32, tag=f"sims{tag}")
        nc.vector.tensor_add(simsh, sims_raw[:, c0 : c0 + nch],
                             maskneg_all[:, b, c0 : c0 + nch])
        maxcol = smallp.tile([P, 1], mybir.dt.float32, tag=f"maxcol{tag}")
        nc.vector.reduce_max(maxcol, simsh, axis=mybir.AxisListType.X)
        maxall = smallp.tile([P, 1], mybir.dt.float32, tag=f"maxall{tag}")
        nc.gpsimd.partition_all_reduce(maxall, maxcol, channels=P,
                                       reduce_op=bass_isa.ReduceOp.max)
        negmax = smallp.tile([P, 1], mybir.dt.float32, tag=f"negmax{tag}")
        nc.scalar.activation(negmax, maxall, mybir.ActivationFunctionType.Copy,
                             scale=-1.0)
        expw = smallp.tile([P, nch], mybir.dt.bfloat16, tag=f"expw{tag}")
        expsum = smallp.tile([P, 1], mybir.dt.float32, tag=f"expsum{tag}")
        nc.scalar.activation(expw, simsh, mybir.ActivationFunctionType.Exp,
                             bias=negmax, scale=1.0, accum_out=expsum)
        lsum = smallp.tile([P, 1], mybir.dt.float32, tag=f"lsum{tag}")
        nc.gpsimd.partition_all_reduce(lsum, expsum, channels=P,
                                       reduce_op=bass_isa.ReduceOp.add)
        return maxall, expw, lsum

    def einsum2_span(prod_bf, expw, c0, nch):
        ctxps = psum_c.tile([1, d], mybir.dt.float32, tag="ctxps")
        for i in range(nch):
            for h0, hw in d_segs:
                nc.tensor.matmul(
                    ctxps[:, h0 : h0 + hw],
                    lhsT=expw[:, i : i + 1],
                    rhs=prod_bf[:, c0 + i, h0 : h0 + hw],
                    start=(i == 0),
                    stop=(i == nch - 1),
                )
        return ctxps

    hh = ch // 2
    half_q = hh // dq

    for b in range(bpc):
        last = b == bpc - 1
        enc3 = enc[b].rearrange("(p c) d -> p c d", p=P)
        dec_b = dec_tiles.pop(b)
        prod_bf = prodp.tile([P, ch, d], mybir.dt.bfloat16, tag="prod_bf")
        sims_raw = smallp.tile([P, ch], mybir.dt.float32, tag="sims_raw")
        halfA = None

        for q in range(n_dma):
            if last and q == n_dma - 1 and dq > 1:
                # final group of the final example: single-chunk DMAs so the
                # DVE can start on chunk ch-2 while ch-1 is still in flight
                tiles = []
                for cc in range(dq):
                    c = q * dq + cc
                    enc_1 = encp.tile([P, 1, d], mybir.dt.float32,
                                      tag="enc_last", bufs=2)
                    nc.sync.dma_start(out=enc_1, in_=enc3[:, c : c + 1, :])
                    tiles.append(enc_1)
                for cc in range(dq):
                    c = q * dq + cc
                    nc.vector.scalar_tensor_tensor(
                        out=prod_bf[:, c, :],
                        in0=tiles[cc][:, 0, :],
                        scalar=1.0,
                        in1=dec_b,
                        op0=mybir.AluOpType.mult,
                        op1=mybir.AluOpType.mult,
                        accum_out=sims_raw[:, c : c + 1],
                    )
                continue
            enc_q = encp.tile([P, dq, d], mybir.dt.float32, tag="enc")
            nc.sync.dma_start(out=enc_q, in_=enc3[:, q * dq : (q + 1) * dq, :])
            if q == 0 and b + 2 < bpc:
                dec_tiles[b + 2] = load_dec(b + 2)
            if last and q == half_q:
                # last example: half A's softmax + context matmuls run while
                # half B is still streaming, so the end-of-kernel tail only
                # holds half B's work (and the PE clock arrives warm)
                mA, expwA, lsumA = softmax_span(sims_raw, b, 0, hh, "A")
                ctxA = einsum2_span(prod_bf, expwA, 0, hh)
                halfA = (mA, lsumA, ctxA)
            for cc in range(dq):
                c = q * dq + cc
                nc.vector.scalar_tensor_tensor(
                    out=prod_bf[:, c, :],
                    in0=enc_q[:, cc, :],
                    scalar=1.0,
                    in1=dec_b,
                    op0=mybir.AluOpType.mult,
                    op1=mybir.AluOpType.mult,
                    accum_out=sims_raw[:, c : c + 1],
                )
            if pending:
                if q >= 1 and "ctxps" not in pending:
                    flush_einsum2()
                elif q >= 3 and "invl" not in pending:
                    flush_recip()
                elif q >= 4 and "invl" in pending:
                    flush_epilogue()

        if not last:
            # ---- whole-example softmax: mask, max, exp (+bf16 weights + sum)
            maxall, expw_bf, lsum = softmax_span(sims_raw, b, 0, ch, "F")
            pending.update(b=b, prod_bf=prod_bf, expw_bf=expw_bf, lsum=lsum)

    # ---- last example's half-B tail + flash-style combine of the halves:
    # ctx = (sA*ctxA + sB*ctxB) / (sA*LA + sB*LB),  sH = exp(mH - max(mA,mB))
    b = bpc - 1
    mA, lsumA, ctxA = halfA
    mB, expwB, lsumB = softmax_span(sims_raw, b, hh, hh, "B")
    ctxB = einsum2_span(prod_bf, expwB, hh, hh)

    m = smallp.tile([P, 1], mybir.dt.float32, tag="m")
    nc.vector.tensor_max(m, mA, mB)
    negm = smallp.tile([P, 1], mybir.dt.float32, tag="negm")
    nc.scalar.activation(negm, m, mybir.ActivationFunctionType.Copy, scale=-1.0)
    sA = smallp.tile([P, 1], mybir.dt.float32, tag="sA")
    nc.scalar.activation(sA, mA, mybir.ActivationFunctionType.Exp, bias=negm)
    sB = smallp.tile([P, 1], mybir.dt.float32, tag="sB")
    nc.scalar.activation(sB, mB, mybir.ActivationFunctionType.Exp, bias=negm)
    t1 = smallp.tile([P, 1], mybir.dt.float32, tag="t1")
    nc.vector.tensor_scalar_mul(t1, lsumB, sB)
    ltot = smallp.tile([P, 1], mybir.dt.float32, tag="ltot")
    nc.vector.scalar_tensor_tensor(out=ltot, in0=lsumA, scalar=sA, in1=t1,
                                   op0=mybir.AluOpType.mult,
                                   op1=mybir.AluOpType.add)
    invt = smallp.tile([P, 1], mybir.dt.float32, tag="invt")
    nc.vector.reciprocal(invt, ltot)
    wA = smallp.tile([P, 1], mybir.dt.float32, tag="wA")
    nc.vector.tensor_scalar_mul(wA, sA, invt)
    wB = smallp.tile([P, 1], mybir.dt.float32, tag="wB")
    nc.vector.tensor_scalar_mul(wB, sB, invt)

    u = outp.tile([1, d], mybir.dt.float32, tag="u")
    nc.scalar.activation(u, ctxA, mybir.ActivationFunctionType.Copy,
                         scale=wA[0:1, :])
    ctx_sb = outp.tile([1, d], mybir.dt.float32, tag="ctx_sb_last")
    nc.vector.scalar_tensor_tensor(out=ctx_sb, in0=ctxB,
                                   scalar=wB[0:1, :], in1=u,
                                   op0=mybir.AluOpType.mult,
                                   op1=mybir.AluOpType.add)
    nc.scalar.dma_start(out=out[b : b + 1, :], in_=ctx_sb)

    # flush example bpc-2's deferred epilogue
    if pending:
        if "ctxps" not in pending:
            flush_einsum2()
        if "invl" not in pending:
            flush_recip()
        flush_epilogue()


def build_nc(bpc: int = BPC, s: int = S, d: int = D, dq: int = 2):
    nc = bacc.Bacc("TRN2", target_bir_lowering=False, debug=False)
    enc = nc.dram_tensor("enc_out", [bpc, s, d], mybir.dt.float32,
                         kind="ExternalInput").ap()
    msk = nc.dram_tensor("enc_mask", [bpc, s], mybir.dt.uint8,
                         kind="ExternalInput").ap()
    dec = nc.dram_tensor("dec_hid", [bpc, d], mybir.dt.float32,
                         kind="ExternalInput").ap()
    out = nc.dram_tensor("context", [bpc, d], mybir.dt.float32,
                         kind="ExternalOutput").ap()
    with tile.TileContext(nc) as tc, ExitStack() as ctx:
        build_kernel_body(ctx, tc, enc, msk, dec, out, bpc, s, d, dq)
    nc.compile()
    return nc


_NC_CACHE = {}


def _get_nc():
    if "nc" not in _NC_CACHE:
        _NC_CACHE["nc"] = build_nc()
    return _NC_CACHE["nc"]


def run_sharded(enc_mask, enc_out, dec_hid, trace=False, **kw):
    """Shard over batch, run on 8 cores, return (full_output, BassKernelResults)."""
    nc = _get_nc()
    enc_mask = np.ascontiguousarray(enc_mask).astype(np.uint8)
    enc_out = np.ascontiguousarray(enc_out, dtype=np.float32)
    dec_hid = np.ascontiguousarray(dec_hid, dtype=np.float32)
    in_maps = [
        {
            "enc_mask": enc_mask[i * BPC : (i + 1) * BPC],
            "enc_out": enc_out[i * BPC : (i + 1) * BPC],
            "dec_hid": dec_hid[i * BPC : (i + 1) * BPC],
        }
        for i in range(N_CORES)
    ]
    res = run_bass_kernel_spmd(nc, in_maps, core_ids=list(range(N_CORES)),
                               trace=trace, **kw)
    full = np.concatenate([r["context"] for r in res.results], axis=0)
    # The device computes sum_s w[s] * (enc[s,:]*dec) = dec .* context;
    # undo the dec factor here.
    full = full / dec_hid
    return full, res


def kernel(enc_mask, enc_out, dec_hid):
    full, _ = run_sharded(enc_mask, enc_out, dec_hid)
    return full.astype(np.float32)


# revision 23
# speedup vs baseline: 1.1734x; 1.0746x over previous
"""Luong attention kernel for Trainium2 (Bass/Tile), batch-parallel over 8 NeuronCores.

Problem (per full input):
    enc_mask [64, 2048] bool, enc_out [64, 2048, 1024] f32, dec_hid [64, 1024] f32
    sims    = einsum('bsd,bd->bs', enc_out, dec_hid); masked -> -inf
    attn    = softmax(sims, axis=1)
    context = einsum('bs,bsd->bd', attn, enc_out)

Strategy: pure data parallelism -- batch dim 64 split 8 ways (8 examples per
core).  Per core, enc_out (64 MB) is streamed through SBUF exactly once on a
dedicated DMA queue (SP/HWDGE) that carries nothing else, so the stream never
stalls behind dependent transfers.  Per 128-row chunk of s:

  * einsum1 (contract d): ONE fused DVE scalar_tensor_tensor per chunk --
    prod_bf (bf16 product, einsum2's moving operand) and the score sum
    (accum_out) come out of a single pass.  No second bulk pass on any
    engine.  (tensor_tensor_reduce would also fold the mask in, but that
    opcode wedges the HW; scalar_tensor_tensor lowers to a standard
    instruction and a tiny [128,16] mask add is done on the side.)
  * softmax: DVE free-dim max; GPSIMD partition all-reduce; ScalarE negate +
    Exp with bf16 main output (einsum2's stationary operand) and fused sum;
    GPSIMD all-reduce.
  * einsum2 (contract s): TensorE matmuls accumulated in PSUM; epilogue
    (1/L scale on ScalarE + store on the ACT DMA queue) is deferred one
    example so no engine ever waits on TensorE.
  * the LAST example is split in two halves with independent softmax stats,
    combined flash-style [ctx = (sA*ctxA + sB*ctxB)/(sA*LA + sB*LB)]: half
    A's context matmuls overlap half B's streaming, so the end-of-kernel
    tail only holds half B's work and the PE clock (HAM gate) arrives warm.

The device computes sum_s w*(enc*dec) = dec .* context; a host-side divide
undoes the dec factor.  s is laid out as s = p*CH + c (p = SBUF partition,
c = chunk) so every DMA is fully contiguous per partition.
"""

from contextlib import ExitStack

import numpy as np

import concourse.bacc as bacc
import concourse.bass as bass
import concourse.tile as tile
from concourse import bass_isa, library_config, mybir
from concourse.bass_utils import run_bass_kernel_spmd

B, S, D = 64, 2048, 1024
N_CORES = 8
BPC = B // N_CORES  # examples per core
P = 128  # SBUF partitions

NEG_BIG = -1.0e30


def build_kernel_body(ctx: ExitStack, tc: "tile.TileContext", enc, msk, dec, out,
                      bpc: int, s: int, d: int, dq: int = 2):
    nc = tc.nc
    ch = s // P                     # chunks of 128 s-values per example
    n_dma = ch // dq                # enc DMAs per example
    d_segs = [(h, min(512, d - h)) for h in range(0, d, 512)]

    encp = ctx.enter_context(tc.tile_pool(name="encp", bufs=8))
    prodp = ctx.enter_context(tc.tile_pool(name="prodp", bufs=2))
    decp = ctx.enter_context(tc.tile_pool(name="decp", bufs=3))
    smallp = ctx.enter_context(tc.tile_pool(name="smallp", bufs=2))
    outp = ctx.enter_context(tc.tile_pool(name="outp", bufs=2))
    psum_b = ctx.enter_context(tc.tile_pool(name="psum_b", bufs=2, space="PSUM"))
    psum_c = ctx.enter_context(tc.tile_pool(name="psum_c", bufs=2, space="PSUM"))

    nc.gpsimd.load_library(library_config.attnmlp)

    ones = smallp.tile([1, P], mybir.dt.float32, bufs=1)
    nc.vector.memset(ones, 1.0)

    # ---- masks for all examples: [128, bpc, ch] {0,1} -> {0, -1e30}
    # Preamble DMAs go on the SP queue BEFORE the enc stream: small qAct
    # transfers starve behind the deep qSP backlog of 2 MiB enc DMAs.
    mask_all = smallp.tile([P, bpc, ch], mybir.dt.uint8, tag="mask_all", bufs=1)
    nc.sync.dma_start(out=mask_all, in_=msk.rearrange("b (p c) -> p b c", p=P))
    maskneg_all = smallp.tile([P, bpc, ch], mybir.dt.float32, tag="maskneg_all",
                              bufs=1)
    nc.vector.tensor_scalar_mul(maskneg_all, mask_all, NEG_BIG)

    def load_dec(b, q=None):
        """dec row b broadcast to all 128 partitions (ones-matmul on TensorE,
        PSUM drained to SBUF on the otherwise-idle ScalarE)."""
        dec_row = decp.tile([1, d], mybir.dt.float32, tag="dec_row")
        (q or nc.scalar).dma_start(out=dec_row, in_=dec[b : b + 1, :])
        dec_ps = psum_b.tile([P, d], mybir.dt.float32, tag="dec_ps")
        for h0, hw in d_segs:
            nc.tensor.matmul(dec_ps[:, h0 : h0 + hw], lhsT=ones,
                             rhs=dec_row[:, h0 : h0 + hw], start=True, stop=True)
        dec_b = decp.tile([P, d], mybir.dt.float32, tag="dec_b")
        nc.scalar.activation(dec_b, dec_ps, mybir.ActivationFunctionType.Copy)
        return dec_b

    dec_tiles = {0: load_dec(0, q=nc.sync)}
    if bpc > 1:
        dec_tiles[1] = load_dec(1, q=nc.sync)

    # deferred per-example tail work, flushed at fixed points inside the NEXT
    # example's chunk loop so no engine's in-order stream ever blocks
    pending = {}

    def flush_einsum2():
        ctxps = psum_c.tile([1, d], mybir.dt.float32, tag="ctxps")
        for c in range(ch):
            for h0, hw in d_segs:
                nc.tensor.matmul(
                    ctxps[:, h0 : h0 + hw],
                    lhsT=pending["expw_bf"][:, c : c + 1],
                    rhs=pending["prod_bf"][:, c, h0 : h0 + hw],
                    start=(c == 0),
                    stop=(c == ch - 1),
                )
        pending["ctxps"] = ctxps

    def flush_recip():
        invl = smallp.tile([P, 1], mybir.dt.float32, tag="invl")
        nc.vector.reciprocal(invl, pending["lsum"])
        pending["invl"] = invl

    def flush_epilogue():
        b_ = pending["b"]
        ctx_sb = outp.tile([1, d], mybir.dt.float32, tag="ctx_sb")
        nc.scalar.activation(ctx_sb, pending["ctxps"],
                             mybir.ActivationFunctionType.Copy,
                             scale=pending["invl"][0:1, :])
        nc.scalar.dma_start(out=out[b_ : b_ + 1, :], in_=ctx_sb)
        pending.clear()

    def softmax_span(sims_raw, b, c0, nch, tag):
        """mask + max + all-reduce + negate + exp for chunks [c0, c0+nch)."""
        simsh = smallp.tile([P, nch], mybir.dt.float32, tag=f"sims{tag}")
        nc.vector.tensor_add(simsh, sims_raw[:, c0 : c0 + nch],
                             maskneg_all[:, b, c0 : c0 + nch])
        maxcol = smallp.tile([P, 1], mybir.dt.float32, tag=f"maxcol{tag}")
        nc.vector.reduce_max(maxcol, simsh, axis=mybir.AxisListType.X)
        maxall = smallp.tile([P, 1], mybir.dt.float32, tag=f"maxall{tag}")
        nc.gpsimd.partition_all_reduce(maxall, maxcol, channels=P,
                                       reduce_op=bass_isa.ReduceOp.max)
        negmax = smallp.tile([P, 1], mybir.dt.float32, tag=f"negmax{tag}")
        nc.scalar.activation(negmax, maxall, mybir.ActivationFunctionType.Copy,
                             scale=-1.0)
        expw = smallp.tile([P, nch], mybir.dt.bfloat16, tag=f"expw{tag}")
        expsum = smallp.tile([P, 1], mybir.dt.float32, tag=f"expsum{tag}")
        nc.scalar.activation(expw, simsh, mybir.ActivationFunctionType.Exp,
                             bias=negmax, scale=1.0, accum_out=expsum)
        lsum = smallp.tile([P, 1], mybir.dt.float32, tag=f"lsum{tag}")
        nc.gpsimd.partition_all_reduce(lsum, expsum, channels=P,
                                       reduce_op=bass_isa.ReduceOp.add)
        return maxall, expw, lsum

    def einsum2_span(prod_bf, expw, c0, nch):
        ctxps = psum_c.tile([1, d], mybir.dt.float32, tag="ctxps")
        for i in range(nch):
            for h0, hw in d_segs:
                nc.tensor.matmul(
                    ctxps[:, h0 : h0 + hw],
                    lhsT=expw[:, i : i + 1],
                    rhs=prod_bf[:, c0 + i, h0 : h0 + hw],
                    start=(i == 0),
                    stop=(i == nch - 1),
                )
        return ctxps

    hh = ch // 2
    half_q = hh // dq

    for b in range(bpc):
        last = b == bpc - 1
        enc3 = enc[b].rearrange("(p c) d -> p c d", p=P)
        dec_b = dec_tiles.pop(b)
        prod_bf = prodp.tile([P, ch, d], mybir.dt.bfloat16, tag="prod_bf")
        sims_raw = smallp.tile([P, ch], mybir.dt.float32, tag="sims_raw")
        halfA = None

        for q in range(n_dma):
            if last and q == n_dma - 1 and dq > 1:
                # final group of the final example: single-chunk DMAs so the
                # DVE can start on chunk ch-2 while ch-1 is still in flight
                tiles = []
                for cc in range(dq):
                    c = q * dq + cc
                    enc_1 = encp.tile([P, 1, d], mybir.dt.float32,
                                      tag="enc_last", bufs=2)
                    nc.sync.dma_start(out=enc_1, in_=enc3[:, c : c + 1, :])
                    tiles.append(enc_1)
                for cc in range(dq):
                    c = q * dq + cc
                    nc.vector.scalar_tensor_tensor(
                        out=prod_bf[:, c, :],
                        in0=tiles[cc][:, 0, :],
                        scalar=1.0,
                        in1=dec_b,
                        op0=mybir.AluOpType.mult,
                        op1=mybir.AluOpType.mult,
                        accum_out=sims_raw[:, c : c + 1],
                    )
                # HAM bridge: dummy matmuls gated on the late chunks keep the
                # PE's activity window busy between einsum2-A and einsum2-B,
                # so the tail's context matmuls run at the warm 2.4 GHz clock
                warm = psum_b.tile([P, d], mybir.dt.float32, tag="dec_ps")
                for cc in range(dq):
                    nc.tensor.matmul(warm[:, 0:512], lhsT=ones,
                                     rhs=tiles[cc][0:1, 0, 0:512],
                                     start=True, stop=True)
                continue
            enc_q = encp.tile([P, dq, d], mybir.dt.float32, tag="enc")
            nc.sync.dma_start(out=enc_q, in_=enc3[:, q * dq : (q + 1) * dq, :])
            if q == 0 and b + 2 < bpc:
                dec_tiles[b + 2] = load_dec(b + 2)
            if last and q == half_q:
                # last example: half A's softmax + context matmuls run while
                # half B is still streaming, so the end-of-kernel tail only
                # holds half B's work (and the PE clock arrives warm)
                mA, expwA, lsumA = softmax_span(sims_raw, b, 0, hh, "A")
                ctxA = einsum2_span(prod_bf, expwA, 0, hh)
                halfA = (mA, lsumA, ctxA)
            for cc in range(dq):
                c = q * dq + cc
                nc.vector.scalar_tensor_tensor(
                    out=prod_bf[:, c, :],
                    in0=enc_q[:, cc, :],
                    scalar=1.0,
                    in1=dec_b,
                    op0=mybir.AluOpType.mult,
                    op1=mybir.AluOpType.mult,
                    accum_out=sims_raw[:, c : c + 1],
                )
            if last and q == n_dma - 2:
                warm = psum_b.tile([P, d], mybir.dt.float32, tag="dec_ps")
                for cc in range(dq):
                    nc.tensor.matmul(warm[:, 0:512], lhsT=ones,
                                     rhs=enc_q[0:1, cc, 0:512],
                                     start=True, stop=True)
            if pending:
                if q >= 1 and "ctxps" not in pending:
                    flush_einsum2()
                elif q >= 3 and "invl" not in pending:
                    flush_recip()
                elif q >= 4 and "invl" in pending:
                    flush_epilogue()

        if not last:
            # ---- whole-example softmax: mask, max, exp (+bf16 weights + sum)
            maxall, expw_bf, lsum = softmax_span(sims_raw, b, 0, ch, "F")
            pending.update(b=b, prod_bf=prod_bf, expw_bf=expw_bf, lsum=lsum)

    # ---- last example's half-B tail + flash-style combine of the halves:
    # ctx = (sA*ctxA + sB*ctxB) / (sA*LA + sB*LB),  sH = exp(mH - max(mA,mB))
    b = bpc - 1
    mA, lsumA, ctxA = halfA
    mB, expwB, lsumB = softmax_span(sims_raw, b, hh, hh, "B")
    ctxB = einsum2_span(prod_bf, expwB, hh, hh)

    m = smallp.tile([P, 1], mybir.dt.float32, tag="m")
    nc.vector.tensor_max(m, mA, mB)
    negm = smallp.tile([P, 1], mybir.dt.float32, tag="negm")
    nc.scalar.activation(negm, m, mybir.ActivationFunctionType.Copy, scale=-1.0)
    sA = smallp.tile([P, 1], mybir.dt.float32, tag="sA")
    nc.scalar.activation(sA, mA, mybir.ActivationFunctionType.Exp, bias=negm)
    sB = smallp.tile([P, 1], mybir.dt.float32, tag="sB")
    nc.scalar.activation(sB, mB, mybir.ActivationFunctionType.Exp, bias=negm)
    t1 = smallp.tile([P, 1], mybir.dt.float32, tag="t1")
    nc.vector.tensor_scalar_mul(t1, lsumB, sB)
    ltot = smallp.tile([P, 1], mybir.dt.float32, tag="ltot")
    nc.vector.scalar_tensor_tensor(out=ltot, in0=lsumA, scalar=sA, in1=t1,
                                   op0=mybir.AluOpType.mult,
                                   op1=mybir.AluOpType.add)
    invt = smallp.tile([P, 1], mybir.dt.float32, tag="invt")
    nc.vector.reciprocal(invt, ltot)
    wA = smallp.tile([P, 1], mybir.dt.float32, tag="wA")
    nc.vector.tensor_scalar_mul(wA, sA, invt)
    wB = smallp.tile([P, 1], mybir.dt.float32, tag="wB")
    nc.vector.tensor_scalar_mul(wB, sB, invt)

    u = outp.tile([1, d], mybir.dt.float32, tag="u")
    nc.scalar.activation(u, ctxA, mybir.ActivationFunctionType.Copy,
                         scale=wA[0:1, :])
    ctx_sb = outp.tile([1, d], mybir.dt.float32, tag="ctx_sb_last")
    nc.vector.scalar_tensor_tensor(out=ctx_sb, in0=ctxB,
                                   scalar=wB[0:1, :], in1=u,
                                   op0=mybir.AluOpType.mult,
                                   op1=mybir.AluOpType.add)
    nc.scalar.dma_start(out=out[b : b + 1, :], in_=ctx_sb)

    # flush example bpc-2's deferred epilogue
    if pending:
        if "ctxps" not in pending:
            flush_einsum2()
        if "invl" not in pending:
            flush_recip()
        flush_epilogue()


def build_nc(bpc: int = BPC, s: int = S, d: int = D, dq: int = 2):
    nc = bacc.Bacc("TRN2", target_bir_lowering=False, debug=False)
    enc = nc.dram_tensor("enc_out", [bpc, s, d], mybir.dt.float32,
                         kind="ExternalInput").ap()
    msk = nc.dram_tensor("enc_mask", [bpc, s], mybir.dt.uint8,
                         kind="ExternalInput").ap()
    dec = nc.dram_tensor("dec_hid", [bpc, d], mybir.dt.float32,
                         kind="ExternalInput").ap()
    out = nc.dram_tensor("context", [bpc, d], mybir.dt.float32,
                         kind="ExternalOutput").ap()
    with tile.TileContext(nc) as tc, ExitStack() as ctx:
        build_kernel_body(ctx, tc, enc, msk, dec, out, bpc, s, d, dq)
    nc.compile()
    return nc


_NC_CACHE = {}


def _get_nc():
    if "nc" not in _NC_CACHE:
        _NC_CACHE["nc"] = build_nc()
    return _NC_CACHE["nc"]


def run_sharded(enc_mask, enc_out, dec_hid, trace=False, **kw):
    """Shard over batch, run on 8 cores, return (full_output, BassKernelResults)."""
    nc = _get_nc()
    enc_mask = np.ascontiguousarray(enc_mask).astype(np.uint8)
    enc_out = np.ascontiguousarray(enc_out, dtype=np.float32)
    dec_hid = np.ascontiguousarray(dec_hid, dtype=np.float32)
    in_maps = [
        {
            "enc_mask": enc_mask[i * BPC : (i + 1) * BPC],
            "enc_out": enc_out[i * BPC : (i + 1) * BPC],
            "dec_hid": dec_hid[i * BPC : (i + 1) * BPC],
        }
        for i in range(N_CORES)
    ]
    res = run_bass_kernel_spmd(nc, in_maps, core_ids=list(range(N_CORES)),
                               trace=trace, **kw)
    full = np.concatenate([r["context"] for r in res.results], axis=0)
    # The device computes sum_s w[s] * (enc[s,:]*dec) = dec .* context;
    # undo the dec factor here.
    full = full / dec_hid
    return full, res


def kernel(enc_mask, enc_out, dec_hid):
    full, _ = run_sharded(enc_mask, enc_out, dec_hid)
    return full.astype(np.float32)
